# revision 1
# baseline (speedup 1.0000x reference)
"""Trainium2 Bass kernel for nn_MatrixFactorization (segment_reduce).

Decomposition (8 cores, SPMD, no collectives):
  - Dedup users of the batch -> unique users, sharded 8 ways (upc per core).
  - Host sorts items by cluster id and packs train_label[uniq].T (item axis
    permuted) as fp8e4m3 (labels are exactly 0/1, so fp8 is lossless) in
    partition-major layout [128, 157, upc]; the label stream is split
    round-robin across the THREE DMA rings (SWDGE/Pool, SP, Activation),
    which pipeline independently, into one fully-resident SBUF buffer.
  - The item matrix rides as fp8 hi+lo pairs t2[128, c, 130] = [fp8(T) |
    fp8(T - fp8(T))] (64 dims + ones column per half); hi+lo recovers
    ~bf16 accuracy while keeping 2 bytes/value.
  - All big matmuls are FLIPPED vs the obvious orientation: the wide
    label/onehot operand is the STATIONARY side (lhsT) and the narrow
    item matrix the MOVING side, so PE time scales with 130 columns per
    matmul, not with users/clusters:
      uni:     DoubleRow fp8 pairs: psum[128u, 130] +=
                 sum_i lt[:, 2j+i, b*128:...].T @ t2[:, 2j+i, :]
               (two K-chunks per matmul at 0.5 cycles/row)
      centers: psum[128c, 130] += onehot_c.T @ t2[:, c, :]
    The ones column accumulates num_rel / cluster counts; hi+lo halves
    are summed on DVE at finalize.
  - Items sorted by cluster => each chunk touches one 128-cluster half
    (two only for the single straddle chunk): half-width onehots, one
    center matmul per chunk, and the half-0 center bank closes mid-stream
    so its finalize + pos-center h0 matmuls hide under the stream.
  - user/pos/neg embeddings: GPSIMD indirect row gathers ([128,1]-offset
    slices, the only form walrus codegen handles correctly), spread
    through the pair loop so their descriptor generation hides under the
    label stream.
  - All outputs are bf16 in partition-major DRAM layouts; host unshuffles
    and upcasts.
"""

import numpy as np
import ml_dtypes

import concourse.bass as bass
import concourse.mybir as mybir
import concourse.tile as tile

NUM_USERS = 10000
NUM_ITEMS = 20000
DIM = 64
CLUSTER = 256
BATCH = 8192
NCORES = 8

KCHUNKS = 157            # ceil(20001 / 128)
KPAD = KCHUNKS * 128     # 20096
M = 65                   # 64 dims + ones col
M2 = 2 * M               # hi | lo
NPAIR = KCHUNKS // 2     # 78 DoubleRow pairs + 1 single chunk
# even-sized groups so DoubleRow pairs never wait on two DMAs
GROUP_SIZES = [4, 8, 8, 12, 12, 16, 16, 20, 20, 20, 20, 1]
assert sum(GROUP_SIZES) == KCHUNKS
# ring index per group: 0=Pool(SWDGE) 1=SP 2=Activation. Pool also carries
# t2 + cpn + the gathers, SP carries t1, Act the packed constants; the
# heavier lt share goes to Act.
GROUP_RING = [0, 2, 1, 1, 2, 2, 0, 2, 1, 2, 1, 1]
assert len(GROUP_RING) == len(GROUP_SIZES)


def split_multiwaits(nc):
    """nix-walrus accepts at most ONE sync-wait per instruction; Tile attaches
    many. Hoist all but the last wait onto single-wait NoOps inserted just
    before the instruction, on the same engine."""
    n_split = 0
    for f in nc.m.functions:
        for bb in f.blocks:
            il = list(bb.instructions)
            new = []
            changed = False
            for ins in il:
                si = ins.sync_info
                if si is not None and si.on_wait is not None and len(si.on_wait) > 1:
                    waits = list(si.on_wait)
                    for k, w in enumerate(waits[:-1]):
                        nop = mybir.InstNoOp(
                            name=f"{ins.name}-wsplit{k}", ins=[], outs=[]
                        )
                        nop.engine = ins.engine
                        nop.sync_info = mybir.SyncInfo(on_wait=[w], on_update=[])
                        new.append(nop)
                    ins.sync_info = mybir.SyncInfo(
                        on_wait=waits[-1:], on_update=list(si.on_update or [])
                    )
                    changed = True
                    n_split += 1
                new.append(ins)
            if changed:
                bb.instructions = new
    return n_split


def build_bass(upc: int, nbpc: int, hlist: tuple, straddle: tuple, cq: int):
    """upc: unique users per core; nbpc: batch entries per core.
    hlist[c]: 128-cluster half touched by (sorted) chunk c; straddle[c]: chunk
    also touches half hlist[c]+1; cq: chunk holding item 20000 (centers use
    the t_cl variant there)."""
    f32 = mybir.dt.float32
    bf16 = mybir.dt.bfloat16
    fp8 = mybir.dt.float8e4
    i32 = mybir.dt.int32
    EQ = mybir.AluOpType.is_equal
    MUL = mybir.AluOpType.mult
    MAX = mybir.AluOpType.max
    ADD = mybir.AluOpType.add
    DR = mybir.MatmulPerfMode.DoubleRow

    assert nbpc % 128 == 0
    jg = nbpc // 128          # embedding gather slices per tensor
    nub = -(-upc // 128)      # user blocks
    # dual-fp8 Ldweights needs the SBUF chunk stride to be a multiple of 64
    # bytes; pad the TILE stride only — the DMA carries just upc columns and
    # the ragged last block's matmuls read a narrower slice
    upc_pad = -(-upc // 64) * 64
    nbb = nbpc // 128         # batch blocks (pos/neg centers)

    # chunk -> center-matmul halves
    contrib = [[] for _ in range(KCHUNKS)]
    for c in range(KCHUNKS):
        contrib[c].append(hlist[c])
        if straddle[c]:
            contrib[c].append(hlist[c] + 1)
    chunks_of = {h: [c for c in range(KCHUNKS) if h in contrib[c]]
                 for h in range(2)}
    assert chunks_of[0] and chunks_of[1]
    stop0 = chunks_of[0][-1]

    nc = bass.Bass(trn_type="TRN2")

    # ---- I/O ----
    # lt is partition-major fp8 over SORTED items:
    #   lt[p, c, u] = label[perm[c*128+p], uniq user u]
    LT = nc.dram_tensor("lt", [128, KCHUNKS, upc], fp8, kind="ExternalInput")
    T2 = nc.dram_tensor("t2", [128, KCHUNKS, M2], fp8, kind="ExternalInput")
    T1 = nc.dram_tensor("t1", [128, KCHUNKS, M], bf16, kind="ExternalInput")
    IOTA = nc.dram_tensor("iota256", [128, CLUSTER], bf16, kind="ExternalInput")
    CIDP = nc.dram_tensor("cidp", [128, KCHUNKS + 2], f32, kind="ExternalInput")
    CPN = nc.dram_tensor("cpn", [128, 2 * nbpc], bf16, kind="ExternalInput")
    UT = nc.dram_tensor("ut_bf", [NUM_USERS, DIM], bf16, kind="ExternalInput")
    IT = nc.dram_tensor("it_bf", [NUM_ITEMS + 1, DIM], bf16, kind="ExternalInput")
    IDX = {}
    for nm in ("uidx", "pidx", "nidx"):
        IDX[nm] = nc.dram_tensor(nm, [128, jg], i32, kind="ExternalInput")

    # partition-major bf16 outputs; host unshuffles and upcasts
    UNI = nc.dram_tensor("uni_part", [128, nub, DIM], bf16, kind="ExternalOutput")
    EMB = {}
    for nm in ("ue_out", "pe_out", "ne_out"):
        EMB[nm] = nc.dram_tensor(nm, [128, jg, DIM], bf16, kind="ExternalOutput")
    PCT = nc.dram_tensor("pct_out", [128, nbb, DIM], bf16, kind="ExternalOutput")
    NCT = nc.dram_tensor("nct_out", [128, nbb, DIM], bf16, kind="ExternalOutput")

    with tile.TileContext(nc) as tc:
        with (
            tc.tile_pool(name="const", bufs=1) as cpool,
            tc.tile_pool(name="ohp", bufs=8) as ohpool,
            tc.tile_pool(name="acc", bufs=1, space="PSUM") as accpool,
            tc.tile_pool(name="outp", bufs=3) as outpool,
        ):
            # ---- packed constants FIRST on the Act ring ----
            cidp_sb = cpool.tile([128, KCHUNKS + 2], f32, name="cidp")
            nc.scalar.dma_start(cidp_sb[:], CIDP[:])
            iota_sb = cpool.tile([128, CLUSTER], bf16)
            nc.scalar.dma_start(iota_sb[:], IOTA[:])
            cid_sb = cidp_sb  # cid = [:, 0:KCHUNKS], pcol = [:, KCHUNKS:+2]
            idx_sb = {}
            g_sb = {}
            for nm, h in IDX.items():
                t = cpool.tile([128, jg], i32, name=f"idx_{nm}")
                nc.scalar.dma_start(t[:], h[:])
                idx_sb[nm] = t
                g_sb[nm] = cpool.tile([128, jg, DIM], bf16, name=f"g_{nm}")

            # ---- fully-resident label buffer + t1/t2, streamed on 3 rings:
            # t2 pieces early on Pool (gates uni), t1 on SP (gates centers),
            # labels round-robin with Act carrying the largest share ----
            lt_sb = cpool.tile([128, KCHUNKS, upc_pad], fp8, name="lt_sb")
            t1_sb = cpool.tile([128, KCHUNKS, M], bf16, name="t1_sb")
            t2_sb = cpool.tile([128, KCHUNKS, M2], fp8, name="t2_sb")
            cpn_sb = cpool.tile([128, 2 * nbpc], bf16, name="cpn_sb")

            rings = [nc.gpsimd, nc.sync, nc.scalar]
            c0s = np.cumsum([0] + GROUP_SIZES[:-1]).tolist()
            nc.gpsimd.dma_start(t2_sb[:, 0:24, :], T2[:, 0:24, :])
            nc.sync.dma_start(t1_sb[:, 0:24, :], T1[:, 0:24, :])
            nc.sync.dma_start(t1_sb[:, 24:80, :], T1[:, 24:80, :])
            for g, gs in enumerate(GROUP_SIZES):
                c0 = c0s[g]
                rings[GROUP_RING[g]].dma_start(
                    lt_sb[:, c0 : c0 + gs, 0:upc], LT[:, c0 : c0 + gs, :]
                )
                if g == 0:
                    nc.gpsimd.dma_start(t2_sb[:, 24:72, :], T2[:, 24:72, :])
                    nc.gpsimd.dma_start(t2_sb[:, 72:KCHUNKS, :],
                                        T2[:, 72:KCHUNKS, :])
                    nc.gpsimd.dma_start(cpn_sb[:], CPN[:])
                if g == 2:
                    nc.sync.dma_start(t1_sb[:, 80:KCHUNKS, :],
                                      T1[:, 80:KCHUNKS, :])


            # uni output staging; zeroed up front so the ragged last user
            # block can skip its pad rows
            uni_sb = outpool.tile([128, nub, DIM], bf16, name="uni_sb", bufs=1)
            nc.vector.memset(uni_sb[:], 0.0)

            # ---- psum accumulators: 6 uni + 2 centers = all 8 banks ----
            uni_ps = [accpool.tile([128, 512], f32, name=f"uni_ps{b}")
                      for b in range(nub)]
            cen_ps = [accpool.tile([128, 512], f32, name=f"cen_ps{h}")
                      for h in range(2)]

            # embedding gathers: [128, 1]-offset slices (the only indirect
            # form walrus codegen handles correctly); spread through the
            # pair loop so the SWDGE generation hides under the stream
            gsrc = {"uidx": UT[:], "pidx": IT[:], "nidx": IT[:]}

            def gather_slice(nm, j):
                nc.gpsimd.indirect_dma_start(
                    out=g_sb[nm][:, j, :],
                    out_offset=None,
                    in_=gsrc[nm],
                    in_offset=bass.IndirectOffsetOnAxis(
                        ap=idx_sb[nm][:, j : j + 1], axis=0
                    ),
                )

            gather_slices = [(nm, j) for nm in ("uidx", "pidx", "nidx")
                             for j in range(jg)]

            ohb = {}
            cen_fin = [None, None]
            cen_started = [False, False]
            pc_specs = (("p", 0, PCT), ("n", 1, NCT))

            def emit_cen(c):
                # one (or two, straddle) half-width onehot center matmuls
                # against the narrow bf16 item matrix (which carries the
                # real item-20000 row, unlike t2's uni variant)
                rhs = t1_sb[:, c, :]
                for h in contrib[c]:
                    oh = ohpool.tile([128, 128], bf16, name="oh")
                    nc.vector.tensor_scalar(
                        oh[:], iota_sb[:, h * 128 : (h + 1) * 128],
                        cid_sb[:, c : c + 1], None, EQ,
                    )
                    nc.tensor.matmul(
                        cen_ps[h][:, 0:M], oh[:], rhs,
                        start=not cen_started[h],
                        stop=(c == chunks_of[h][-1]),
                    )
                    cen_started[h] = True

            def finalize_centers(h):
                # max(count,1), reciprocal, scale -> bf16
                cs = outpool.tile([128, M], f32, name=f"cs{h}", bufs=1)
                nc.vector.tensor_copy(cs[:], cen_ps[h][:, 0:M])
                nc.vector.tensor_scalar(
                    cs[:, 64:65], cs[:, 64:65], 1.0, None, MAX
                )
                rc = outpool.tile([128, 1], f32, name=f"crc{h}")
                nc.vector.reciprocal(rc[:], cs[:, 64:65])
                cf = outpool.tile([128, DIM], bf16, name=f"cf{h}", bufs=1)
                nc.vector.tensor_scalar(cf[:], cs[:, 0:DIM], rc[:], None, MUL)
                cen_fin[h] = cf

            def pc_matmuls(key, ps, hs, start):
                for hi, h in enumerate(hs):
                    for b in range(nbb):
                        nc.tensor.matmul(
                            ps[:, b * DIM : (b + 1) * DIM],
                            ohb[(key, h)][:, b * 128 : (b + 1) * 128],
                            cen_fin[h][:],
                            start=(start and hi == 0 and b == 0),
                            stop=(h == 1 and b == nbb - 1),
                        )

            # ---- main compute loop over DoubleRow pairs ----
            for j in range(NPAIR):
                emit_cen(2 * j)
                emit_cen(2 * j + 1)
                for b in range(nub):
                    bw = min(128, upc - b * 128)
                    nc.tensor.matmul(
                        uni_ps[b][0:bw, 0:M2],
                        lt_sb[:, 2 * j : 2 * j + 2, b * 128 : b * 128 + bw],
                        t2_sb[:, 2 * j : 2 * j + 2, :],
                        start=(j == 0), stop=False,
                        perf_mode=DR,
                    )
                if 16 <= j < 16 + len(gather_slices):
                    gather_slice(*gather_slices[j - 16])
                if j == 12:
                    # pos/neg center onehots (batch cluster ids vs partition
                    # index); DVE has slack mid-loop
                    for ki, key in enumerate(("p", "n")):
                        for h in range(2):
                            t = cpool.tile([128, nbpc], bf16,
                                           name=f"ohb_{key}{h}")
                            nc.vector.tensor_scalar(
                                t[:],
                                cpn_sb[:, ki * nbpc : (ki + 1) * nbpc],
                                cidp_sb[:, KCHUNKS + h : KCHUNKS + h + 1],
                                None, EQ,
                            )
                            ohb[(key, h)] = t
                if cen_fin[0] is None and 2 * j + 1 >= stop0 and j >= 10:
                    finalize_centers(0)
                    pc_matmuls("p", cen_ps[0], (0,), start=True)
            # final unpaired chunk
            c = KCHUNKS - 1
            emit_cen(c)
            for b in range(nub):
                bw = min(128, upc - b * 128)
                nc.tensor.matmul(
                    uni_ps[b][0:bw, 0:M2],
                    lt_sb[:, c, b * 128 : b * 128 + bw],
                    t2_sb[:, c, :],
                    start=False, stop=True,
                )

            # ---- tail: half-1 centers, uni finalize, pc close ----
            if cen_fin[0] is None:  # fallback: half 0 closed very late
                finalize_centers(0)
                pc_matmuls("p", cen_ps[0], (0,), start=True)
            finalize_centers(1)

            # uni finalize on DVE (overlaps the pc matmuls on PE):
            # hi+lo merge, reciprocal of num_rel, scale
            u2 = outpool.tile([128, M], f32, name="u2")
            r_all = outpool.tile([128, nub], f32, name="urc_all", bufs=1)
            for b in range(nub):
                bw = min(128, upc - b * 128)
                # DVE may read only ONE non-scalar PSUM input per op
                nc.vector.tensor_copy(u2[0:bw, :], uni_ps[b][0:bw, 0:M])
                nc.vector.tensor_tensor(
                    u2[0:bw, :], u2[0:bw, :], uni_ps[b][0:bw, M:M2], ADD
                )
                nc.vector.reciprocal(r_all[0:bw, b : b + 1], u2[0:bw, 64:65])
                nc.vector.tensor_scalar(
                    uni_sb[0:bw, b, :], u2[0:bw, 0:DIM],
                    r_all[0:bw, b : b + 1], None, MUL,
                )
            nc.sync.dma_start(UNI[:], uni_sb[:])

            pc_matmuls("p", cen_ps[0], (1,), start=False)
            # the neg rep waits for the cen1 bank: its h0+h1 matmuls both
            # run here, after the half-1 finalize frees the bank
            pc_matmuls("n", cen_ps[1], (0, 1), start=True)

            # embedding writebacks (gathers completed mid-loop)
            for eng, nm, out in ((nc.gpsimd, "uidx", EMB["ue_out"]),
                                 (nc.sync, "pidx", EMB["pe_out"]),
                                 (nc.scalar, "nidx", EMB["ne_out"])):
                eng.dma_start(out[:], g_sb[nm][:])

            # pc copies: pos on Activation, neg on DVE (parallel tails)
            for key, ri, out in pc_specs:
                pc_sb = outpool.tile([128, nbb * DIM], bf16, name=f"pc_{key}",
                                     bufs=1)
                if key == "p":
                    nc.scalar.copy(pc_sb[:], cen_ps[ri][:, 0 : nbb * DIM])
                else:
                    nc.vector.tensor_copy(pc_sb[:], cen_ps[ri][:, 0 : nbb * DIM])
                eng = nc.sync if key == "p" else nc.scalar
                eng.dma_start(out[:], pc_sb[:])

    split_multiwaits(nc)
    return nc


# ------------------------- host side -------------------------

def _wrap_idx(idx: np.ndarray) -> np.ndarray:
    """indirect gather layout: element [p, j] = idx[j*128 + p]."""
    n = idx.shape[0]
    return np.ascontiguousarray(idx.astype(np.int32).reshape(n // 128, 128).T)


def _hi_lo(t: np.ndarray):
    """fp8e4m3 hi + residual-lo decomposition (hi+lo ~ bf16 accuracy)."""
    hi = t.astype(ml_dtypes.float8_e4m3)
    lo = (t - hi.astype(np.float32)).astype(ml_dtypes.float8_e4m3)
    return hi, lo


def host_prep(user, pos, neg, cluster_ids, user_table, item_table, train_label):
    user = np.asarray(user).astype(np.int64)
    pos = np.asarray(pos).astype(np.int64)
    neg = np.asarray(neg).astype(np.int64)
    cluster_ids = np.asarray(cluster_ids).astype(np.int64)
    user_table = np.ascontiguousarray(np.asarray(user_table, dtype=np.float32))
    item_table = np.ascontiguousarray(np.asarray(item_table, dtype=np.float32))
    train_label = np.asarray(train_label, dtype=np.float32)

    uniq, inverse = np.unique(user, return_inverse=True)
    nu = len(uniq)
    upc = -(-nu // (NCORES * 2)) * 2  # per-core users, mult of 2
    upad = upc * NCORES
    uu = np.concatenate([uniq, np.full(upad - nu, uniq[0], dtype=uniq.dtype)])

    # sort items by cluster id (stable) so each chunk touches one half
    perm = np.argsort(cluster_ids, kind="stable").astype(np.int64)
    cs_cid = cluster_ids[perm]                 # ascending
    q = int(np.nonzero(perm == NUM_ITEMS)[0][0])
    cq = q // 128

    hlist, straddle = [], []
    for c in range(KCHUNKS):
        lo = c * 128
        hi = min(lo + 127, NUM_ITEMS)
        h0 = int(cs_cid[lo]) // 128
        h1 = int(cs_cid[hi]) // 128
        hlist.append(h0)
        straddle.append(h1 != h0)
    hlist, straddle = tuple(hlist), tuple(straddle)

    # t_aug over sorted items: fp8 hi|lo halves t2 [128, KCHUNKS, 130] for
    # the uni DoubleRow matmuls, narrow bf16 t1 [128, KCHUNKS, 65] for the
    # center matmuls; item 20000's dims zeroed for uni
    t_aug = np.zeros((KPAD, M), np.float32)
    t_aug[: NUM_ITEMS + 1, :DIM] = item_table[perm]
    t_aug[q, :DIM] = 0.0
    t_aug[: NUM_ITEMS + 1, DIM] = 1.0
    hi, lo = _hi_lo(t_aug)
    t2 = np.concatenate([hi.reshape(KCHUNKS, 128, M),
                         lo.reshape(KCHUNKS, 128, M)], axis=2)
    t2 = np.ascontiguousarray(t2.transpose(1, 0, 2))  # [128, KCHUNKS, 130]

    # centers variant: real item 20000 dims (t1 + the cq-chunk override)
    t_cen = t_aug.copy()
    t_cen[q, :DIM] = item_table[NUM_ITEMS]
    t1 = np.ascontiguousarray(
        t_cen.reshape(KCHUNKS, 128, M).transpose(1, 0, 2)
    ).astype(ml_dtypes.bfloat16)

    iota256 = np.broadcast_to(
        np.arange(CLUSTER, dtype=np.float32), (128, CLUSTER)
    ).astype(ml_dtypes.bfloat16)
    cid_pm = np.full((KPAD,), -1.0, np.float32)
    cid_pm[: NUM_ITEMS + 1] = cs_cid.astype(np.float32)
    cid_pm = cid_pm.reshape(KCHUNKS, 128).T
    pcol = (np.arange(128, dtype=np.float32)[:, None]
            + np.array([0.0, 128.0], np.float32)[None, :])
    cidp = np.ascontiguousarray(np.concatenate([cid_pm, pcol], axis=1))

    cpos = cluster_ids[pos].astype(np.float32)
    cneg = cluster_ids[neg].astype(np.float32)

    nbpc = BATCH // NCORES
    shared = {
        "t1": t1,
        "t2": t2,
        "iota256": iota256,
        "cidp": cidp,
        "ut_bf": user_table.astype(ml_dtypes.bfloat16),
        "it_bf": item_table.astype(ml_dtypes.bfloat16),
    }
    in_maps = []
    for c in range(NCORES):
        rows = uu[c * upc : (c + 1) * upc]
        gathered = train_label[rows]  # [upc, 20001] f32
        lt = np.zeros((KPAD, upc), ml_dtypes.float8_e4m3)
        lt[: NUM_ITEMS + 1, :] = gathered.T[perm].astype(ml_dtypes.float8_e4m3)
        ltpm = np.ascontiguousarray(
            lt.reshape(KCHUNKS, 128, upc).transpose(1, 0, 2)
        )
        bs = slice(c * nbpc, (c + 1) * nbpc)
        m = dict(shared)
        m["lt"] = ltpm
        m["uidx"] = _wrap_idx(user[bs])
        m["pidx"] = _wrap_idx(pos[bs])
        m["nidx"] = _wrap_idx(neg[bs])
        m["cpn"] = np.ascontiguousarray(np.broadcast_to(
            np.concatenate([cpos[bs], cneg[bs]])[None, :], (128, 2 * nbpc)
        )).astype(ml_dtypes.bfloat16)
        in_maps.append(m)

    meta = {"upc": upc, "nbpc": nbpc, "nu": nu, "inverse": inverse,
            "hlist": hlist, "straddle": straddle, "cq": cq}
    return in_maps, meta


def _unshuffle_pm(arr):
    """[128, nblk, 64] partition-major -> [nblk*128, 64] row-major f32."""
    arr = np.asarray(arr, dtype=np.float32)
    return np.ascontiguousarray(arr.transpose(1, 0, 2)).reshape(-1, arr.shape[2])


def assemble(results, meta):
    inverse = meta["inverse"]
    upc = meta["upc"]
    uni_unique = np.concatenate(
        [_unshuffle_pm(r["uni_part"])[:upc] for r in results], axis=0
    )
    uni = uni_unique[inverse]
    ue = np.concatenate([_unshuffle_pm(r["ue_out"]) for r in results], axis=0)
    pe = np.concatenate([_unshuffle_pm(r["pe_out"]) for r in results], axis=0)
    ne = np.concatenate([_unshuffle_pm(r["ne_out"]) for r in results], axis=0)
    pc = np.concatenate(
        [_unshuffle_pm(r["pct_out"].reshape(128, -1, DIM)) for r in results],
        axis=0,
    )
    ncen = np.concatenate(
        [_unshuffle_pm(r["nct_out"].reshape(128, -1, DIM)) for r in results],
        axis=0,
    )
    return ue, pe, ne, pc, ncen, uni


_CACHE = {}


def build_from_meta(meta):
    return build_bass(meta["upc"], meta["nbpc"], meta["hlist"],
                      meta["straddle"], meta["cq"])


def _run(in_maps, meta, trace=False):
    from concourse.bass_utils import run_bass_kernel_spmd

    key = (meta["upc"], meta["nbpc"], meta["hlist"], meta["straddle"],
           meta["cq"])
    if key not in _CACHE:
        _CACHE[key] = build_from_meta(meta)
    nc = _CACHE[key]
    res = run_bass_kernel_spmd(
        nc, in_maps, core_ids=list(range(NCORES)), trace=trace
    )
    return res


def kernel(user, pos, neg, cluster_ids, user_table, item_table, train_label):
    """Full (unsharded) inputs -> full outputs, computed on 8 NeuronCores."""
    in_maps, meta = host_prep(
        user, pos, neg, cluster_ids, user_table, item_table, train_label
    )
    res = _run(in_maps, meta)
    return assemble(res.results, meta)



# revision 10
# speedup vs baseline: 1.1844x; 1.1844x over previous
"""Trainium2 Bass kernel for nn_MatrixFactorization (segment_reduce).

Decomposition (8 cores, SPMD, no collectives):
  - Dedup users of the batch -> unique users, sharded 8 ways (upc per core).
  - Host sorts items by cluster id and packs train_label[uniq].T (item axis
    permuted) as fp8e4m3 (labels are exactly 0/1, so fp8 is lossless) in
    partition-major layout [128, 157, upc]; label chunks are split across
    the THREE DMA rings (SP, Activation, Pool/SWDGE) by a greedy
    balance-aware schedule.
  - The item matrix rides as fp8 hi+lo pairs t2i[128, c, 2, 64] (hi | the
    residual T - fp8(T)); a single DoubleRow matmul per chunk contracts
    BOTH halves (the lhsT k-tile axis is a stride-0 broadcast of the same
    label columns), so hi+lo lands pre-summed in one 64-wide psum:
      uni:  psum[128u, 64] += lt[:, c, blk].T @ hi + lt.T @ lo
    at 0.5 cycles/row with no merge pass.
  - Cluster centers use the same trick with HOST-PACKED fp8 onehots:
    items are sorted by cluster, so each chunk's clusters fit a 32-wide
    aligned window; OH[128, c, 32] holds the indicator columns and one
    DR matmul per chunk accumulates into cen_ps[half][win:win+32, 0:64].
    Rare window-crossing chunks get a second matmul from a small OH2
    side buffer. start/stop flags per aligned window come from host
    metadata.
  - num_rel and per-cluster counts are computed EXACTLY on the host
    (row sums / bincount of integers); the device ships raw sums and the
    host divides, so there is no on-chip finalize chain.
  - pos/neg centers: host-packed fp8 batch onehots (OHB) matmul the raw
    center sums (bf16 copy of psum); host divides by counts[cid].
  - user/pos/neg embeddings: BATCHED GPSIMD indirect row gathers (one
    call per table; multi-column [128, jg] offsets) - SWDGE descriptor
    generation is ~1.3us per call instead of ~1us per 128 rows.
  - All outputs are partition-major; host unshuffles, divides, upcasts.
"""

import numpy as np
import ml_dtypes

import concourse.bass as bass
import concourse.mybir as mybir
import concourse.tile as tile

NUM_USERS = 10000
NUM_ITEMS = 20000
DIM = 64
CLUSTER = 256
BATCH = 8192
NCORES = 8

KCHUNKS = 157            # ceil(20001 / 128)
KPAD = KCHUNKS * 128     # 20096
WIN = 32                 # aligned cluster window width for center matmuls


def split_multiwaits(nc):
    """nix-walrus accepts at most ONE sync-wait per instruction; Tile attaches
    many. Hoist all but the last wait onto single-wait NoOps inserted just
    before the instruction, on the same engine."""
    n_split = 0
    for f in nc.m.functions:
        for bb in f.blocks:
            il = list(bb.instructions)
            new = []
            changed = False
            for ins in il:
                si = ins.sync_info
                if si is not None and si.on_wait is not None and len(si.on_wait) > 1:
                    waits = list(si.on_wait)
                    for k, w in enumerate(waits[:-1]):
                        nop = mybir.InstNoOp(
                            name=f"{ins.name}-wsplit{k}", ins=[], outs=[]
                        )
                        nop.engine = ins.engine
                        nop.sync_info = mybir.SyncInfo(on_wait=[w], on_update=[])
                        new.append(nop)
                    ins.sync_info = mybir.SyncInfo(
                        on_wait=waits[-1:], on_update=list(si.on_update or [])
                    )
                    changed = True
                    n_split += 1
                new.append(ins)
            if changed:
                bb.instructions = new
    return n_split


def _bcast_ktile(ap):
    """Insert a stride-0 k-tile axis: [128, n] -> [128, 2(bcast), n].
    DoubleRow then contracts the SAME stationary columns against both
    rhs k-tiles (hi and lo halves of the item matrix)."""
    return bass.AP(ap.tensor, ap.offset, [ap.ap[0], [0, 2], ap.ap[1]])


def build_bass(upc: int, nbpc: int, emit: tuple, ncross: int):
    """upc: unique users per core; nbpc: batch entries per core.
    emit[c]: tuple of center-matmul descriptors for (sorted) chunk c, each
    (src, idx, h, base, start, stop): src 0 -> OH[:, c, :], 1 -> OH2[:, idx, :];
    accumulate into cen_ps[h][base:base+WIN, 0:DIM]."""
    f32 = mybir.dt.float32
    bf16 = mybir.dt.bfloat16
    fp8 = mybir.dt.float8e4
    i32 = mybir.dt.int32
    DR = mybir.MatmulPerfMode.DoubleRow

    assert nbpc % 128 == 0
    jg = nbpc // 128          # embedding gather slices per tensor
    nub = -(-upc // 128)      # user blocks
    # dual-fp8 Ldweights needs the SBUF chunk stride to be a multiple of 64
    # bytes; pad the TILE stride only - the DMA carries just upc columns and
    # the ragged last block's matmuls read a narrower slice
    upc_pad = -(-upc // 64) * 64
    nbb = nbpc // 128         # batch blocks (pos/neg centers)

    # last chunk of each cen bank (for the mid-stream finalize of bank 0)
    bank_last = {0: -1, 1: -1}
    for c in range(KCHUNKS):
        for (_s, _i, h, _b, _st, _sp) in emit[c]:
            bank_last[h] = max(bank_last[h], c)
    assert bank_last[0] >= 0 and bank_last[1] >= 0

    nc = bass.Bass(trn_type="TRN2")

    # ---- I/O ----
    LT = nc.dram_tensor("lt", [128, KCHUNKS, upc], fp8, kind="ExternalInput")
    T2I = nc.dram_tensor("t2i", [128, KCHUNKS, 2, DIM], fp8, kind="ExternalInput")
    OH = nc.dram_tensor("oh", [128, KCHUNKS, WIN], fp8, kind="ExternalInput")
    OH2 = nc.dram_tensor("oh2", [128, ncross, WIN], fp8, kind="ExternalInput")
    OHB = nc.dram_tensor("ohb", [128, 4, nbpc], fp8, kind="ExternalInput")
    UIDX = nc.dram_tensor("uidx", [128, jg], i32, kind="ExternalInput")
    PNIDX = nc.dram_tensor("pnidx", [128, 2 * jg], i32, kind="ExternalInput")
    UT = nc.dram_tensor("ut_bf", [NUM_USERS, DIM], bf16, kind="ExternalInput")
    IT = nc.dram_tensor("it_bf", [NUM_ITEMS + 1, DIM], bf16, kind="ExternalInput")

    # partition-major outputs; host unshuffles / divides / upcasts
    UNI = nc.dram_tensor("uni_part", [128, nub, DIM], f32, kind="ExternalOutput")
    UE = nc.dram_tensor("ue_out", [128, jg, DIM], bf16, kind="ExternalOutput")
    PNE = nc.dram_tensor("pne_out", [128, 2 * jg, DIM], bf16, kind="ExternalOutput")
    PCT = nc.dram_tensor("pct_out", [128, nbb * DIM], f32, kind="ExternalOutput")
    NCT = nc.dram_tensor("nct_out", [128, nbb * DIM], f32, kind="ExternalOutput")

    # ---- label-chunk DMA schedule: greedy balance across the 3 rings ----
    # Ranges of chunks in process order; fixed payloads with due-chunks.
    CHUNK_NS = upc * 128 / 360.0          # transfer ns per label chunk
    ranges = []
    c0 = 0
    for sz in [4, 6, 8, 8, 8, 12, 12, 12, 12, 12, 16, 16, 16, 15]:
        ranges.append((c0, min(c0 + sz, KCHUNKS)))
        c0 += sz
    assert c0 >= KCHUNKS and ranges[-1][1] == KCHUNKS

    # fixed jobs: (ring, due_chunk, kind, payload_ns)
    T2I_CH_NS = 2 * DIM * 128 / 360.0
    OH_CH_NS = WIN * 128 / 360.0
    fixed = {
        0: [(0, "t2i_0_24", 24 * T2I_CH_NS),
            (18, "t2i_24_96", 72 * T2I_CH_NS),
            (88, "t2i_96_157", 61 * T2I_CH_NS)],
        1: [(0, "oh_0_64", 64 * OH_CH_NS + ncross * OH_CH_NS + 2000),
            (56, "oh_64_157", 93 * OH_CH_NS),
            (76, "ohb", 4 * nbpc * 128 / 360.0)],
        2: [(20, "gather_u", jg * 128 * 128 * 2 / 360.0 * 2 + 1400),
            (48, "gather_pn", 2 * jg * 128 * 128 * 2 / 360.0 * 2 + 1700)],
    }
    clock = {0: 0.0, 1: 0.0, 2: 0.0}
    sched = {0: [], 1: [], 2: []}         # per-ring ordered job list
    fi = {0: 0, 1: 0, 2: 0}
    for (a, b) in ranges:
        for r in range(3):
            while fi[r] < len(fixed[r]) and fixed[r][fi[r]][0] <= b:
                _due, kind, ns = fixed[r][fi[r]]
                sched[r].append(("fixed", kind))
                clock[r] += ns
                fi[r] += 1
        r = min(range(3), key=lambda q: clock[q])
        sched[r].append(("lt", (a, b)))
        clock[r] += (b - a) * CHUNK_NS
    for r in range(3):
        while fi[r] < len(fixed[r]):
            sched[r].append(("fixed", fixed[r][fi[r]][1]))
            fi[r] += 1

    with tile.TileContext(nc) as tc:
        with (
            tc.tile_pool(name="const", bufs=1) as cpool,
            tc.tile_pool(name="acc", bufs=1, space="PSUM") as accpool,
            tc.tile_pool(name="outp", bufs=1) as outpool,
        ):
            uni_sb = cpool.tile([128, nub, DIM], f32, name="uni_sb")
            nc.vector.memset(uni_sb[:], 0.0)
            lt_sb = cpool.tile([128, KCHUNKS, upc_pad], fp8, name="lt_sb")
            t2i_sb = cpool.tile([128, KCHUNKS, 2, DIM], fp8, name="t2i_sb")
            oh_sb = cpool.tile([128, KCHUNKS, WIN], fp8, name="oh_sb")
            oh2_sb = cpool.tile([128, ncross, WIN], fp8, name="oh2_sb")
            ohb_sb = cpool.tile([128, 4, nbpc], fp8, name="ohb_sb")
            uidx_sb = cpool.tile([128, jg], i32, name="uidx_sb")
            pnidx_sb = cpool.tile([128, 2 * jg], i32, name="pnidx_sb")
            gu_sb = cpool.tile([128, jg, DIM], bf16, name="gu_sb")
            gpn_sb = cpool.tile([128, 2 * jg, DIM], bf16, name="gpn_sb")

            rings = [nc.sync, nc.scalar, nc.gpsimd]

            def issue_fixed(r, kind):
                eng = rings[r]
                if kind == "t2i_0_24":
                    eng.dma_start(t2i_sb[:, 0:24], T2I[:, 0:24])
                elif kind == "t2i_24_96":
                    eng.dma_start(t2i_sb[:, 24:96], T2I[:, 24:96])
                elif kind == "t2i_96_157":
                    eng.dma_start(t2i_sb[:, 96:KCHUNKS], T2I[:, 96:KCHUNKS])
                elif kind == "oh_0_64":
                    eng.dma_start(uidx_sb[:], UIDX[:])
                    eng.dma_start(pnidx_sb[:], PNIDX[:])
                    eng.dma_start(oh2_sb[:], OH2[:])
                    eng.dma_start(oh_sb[:, 0:64], OH[:, 0:64])
                elif kind == "oh_64_157":
                    eng.dma_start(oh_sb[:, 64:KCHUNKS], OH[:, 64:KCHUNKS])
                elif kind == "ohb":
                    eng.dma_start(ohb_sb[:], OHB[:])
                elif kind == "gather_u":
                    nc.gpsimd.indirect_dma_start(
                        out=gu_sb[:], out_offset=None, in_=UT[:],
                        in_offset=bass.IndirectOffsetOnAxis(ap=uidx_sb[:], axis=0),
                    )
                elif kind == "gather_pn":
                    nc.gpsimd.indirect_dma_start(
                        out=gpn_sb[:], out_offset=None, in_=IT[:],
                        in_offset=bass.IndirectOffsetOnAxis(ap=pnidx_sb[:], axis=0),
                    )
                else:
                    raise AssertionError(kind)

            # issue the DMA schedule; ring issue order == process order per ring
            for r in range(3):
                for job, arg in sched[r]:
                    if job == "fixed":
                        issue_fixed(r, arg)
                    else:
                        a, b = arg
                        rings[r].dma_start(
                            lt_sb[:, a:b, 0:upc], LT[:, a:b, :]
                        )

            # ---- psum: 6 uni banks + 2 center banks ----
            uni_ps = [accpool.tile([128, 512], f32, name=f"uni_ps{b}")
                      for b in range(nub)]
            cen_ps = [accpool.tile([128, 512], f32, name=f"cen_ps{h}")
                      for h in range(2)]

            cen_raw = [None, None]

            def copy_cen(h):
                t = outpool.tile([128, DIM], bf16, name=f"cen_raw{h}")
                nc.vector.tensor_copy(t[:], cen_ps[h][:, 0:DIM])
                cen_raw[h] = t

            def pc_matmuls(key, ps, hs, start):
                # gather raw center sums for the batch: ohb.T @ cen_raw
                for hi_, h in enumerate(hs):
                    oi = {"p": 0, "n": 2}[key] + h
                    for b in range(nbb):
                        nc.tensor.matmul(
                            ps[:, b * DIM:(b + 1) * DIM],
                            ohb_sb[:, oi, b * 128:(b + 1) * 128],
                            cen_raw[h][:],
                            start=(start and hi_ == 0 and b == 0),
                            stop=(h == 1 and b == nbb - 1),
                            skip_group_check=True,
                        )

            # ---- main loop over chunks ----
            for c in range(KCHUNKS):
                for (src, idx, h, base, st, sp) in emit[c]:
                    lhs = oh_sb[:, c, :] if src == 0 else oh2_sb[:, idx, :]
                    nc.tensor.matmul(
                        cen_ps[h][base:base + WIN, 0:DIM],
                        _bcast_ktile(lhs),
                        t2i_sb[:, c, :, :],
                        start=st, stop=sp,
                        perf_mode=DR, skip_group_check=True,
                        tile_position=(0, base),
                    )
                for b in range(nub):
                    bw = min(128, upc - b * 128)
                    nc.tensor.matmul(
                        uni_ps[b][0:bw, 0:DIM],
                        _bcast_ktile(lt_sb[:, c, b * 128:b * 128 + bw]),
                        t2i_sb[:, c, :, :],
                        start=(c == 0), stop=(c == KCHUNKS - 1),
                        perf_mode=DR,
                    )
                if c == bank_last[0]:
                    # bank 0 closed: free it (DVE copy of the raw sums)
                    copy_cen(0)
                if c == min(bank_last[0] + 8, KCHUNKS - 2):
                    # h0 share of the pos-center gather, a few chunks later
                    # so the in-order PE stream never stalls on the DVE copy
                    pc_matmuls("p", cen_ps[0], (0,), start=True)

            # ---- tail ----
            copy_cen(1)
            pc_matmuls("p", cen_ps[0], (1,), start=False)
            pc_matmuls("n", cen_ps[1], (0, 1), start=True)

            # uni psum -> sbuf (f32), split DVE/Act, then writeback
            # (memset so the ragged last block's pad rows are defined)
            for b in range(nub):
                bw = min(128, upc - b * 128)
                if b % 2 == 0:
                    nc.vector.tensor_copy(uni_sb[0:bw, b, :],
                                          uni_ps[b][0:bw, 0:DIM])
                else:
                    nc.scalar.copy(uni_sb[0:bw, b, :],
                                   uni_ps[b][0:bw, 0:DIM])
            nc.sync.dma_start(UNI[:], uni_sb[:])

            # embedding writebacks (gathers completed mid-loop)
            nc.scalar.dma_start(UE[:], gu_sb[:])
            nc.sync.dma_start(PNE[:], gpn_sb[:])

            # pos/neg center writebacks via sbuf
            pct_sb = outpool.tile([128, nbb * DIM], f32, name="pct_sb")
            nc.vector.tensor_copy(pct_sb[:], cen_ps[0][:, 0:nbb * DIM])
            nc.scalar.dma_start(PCT[:], pct_sb[:])
            nct_sb = outpool.tile([128, nbb * DIM], f32, name="nct_sb")
            nc.scalar.copy(nct_sb[:], cen_ps[1][:, 0:nbb * DIM])
            nc.gpsimd.dma_start(NCT[:], nct_sb[:])

    split_multiwaits(nc)
    return nc


# ------------------------- host side -------------------------

def _wrap_idx(idx: np.ndarray) -> np.ndarray:
    """indirect gather layout: element [p, j] = idx[j*128 + p]."""
    n = idx.shape[0]
    return np.ascontiguousarray(idx.astype(np.int32).reshape(n // 128, 128).T)


def host_prep(user, pos, neg, cluster_ids, user_table, item_table, train_label):
    user = np.asarray(user).astype(np.int64)
    pos = np.asarray(pos).astype(np.int64)
    neg = np.asarray(neg).astype(np.int64)
    cluster_ids = np.asarray(cluster_ids).astype(np.int64)
    user_table = np.ascontiguousarray(np.asarray(user_table, dtype=np.float32))
    item_table = np.ascontiguousarray(np.asarray(item_table, dtype=np.float32))
    train_label = np.asarray(train_label, dtype=np.float32)

    uniq, inverse = np.unique(user, return_inverse=True)
    nu = len(uniq)
    upc = -(-nu // NCORES)
    upad = upc * NCORES
    uu = np.concatenate([uniq, np.full(upad - nu, uniq[0], dtype=uniq.dtype)])

    # sort items by cluster id (stable) so each chunk's clusters sit in a
    # narrow aligned window
    perm = np.argsort(cluster_ids, kind="stable").astype(np.int64)
    cs_cid = cluster_ids[perm]                 # ascending
    q = int(np.nonzero(perm == NUM_ITEMS)[0][0])  # sorted pos of item 20000

    # t2i: fp8 hi + residual-lo of the item matrix over sorted items,
    # k-tile-major [128, KCHUNKS, 2, 64]; the REAL item-20000 row stays
    # (its uni contribution is killed by zeroing its labels instead)
    t_aug = np.zeros((KPAD, DIM), np.float32)
    t_aug[: NUM_ITEMS + 1] = item_table[perm]
    hi = t_aug.astype(ml_dtypes.float8_e4m3)
    lo = (t_aug - hi.astype(np.float32)).astype(ml_dtypes.float8_e4m3)
    t2i = np.stack([hi.reshape(KCHUNKS, 128, DIM),
                    lo.reshape(KCHUNKS, 128, DIM)], axis=2)
    t2i = np.ascontiguousarray(t2i.transpose(1, 0, 2, 3))  # [128, c, 2, 64]

    # center onehots: per chunk a 32-aligned cluster window (plus rare
    # crossing windows in OH2) with start/stop per aligned window
    cid_pm = np.full((KCHUNKS, 128), -1, np.int64)
    cid_pm.reshape(-1)[: NUM_ITEMS + 1] = cs_cid
    oh = np.zeros((KCHUNKS, 128, WIN), ml_dtypes.float8_e4m3)
    oh2_list = []
    touch = {}                 # aligned window base -> list of (c, src, idx)
    for c in range(KCHUNKS):
        real = cid_pm[c][cid_pm[c] >= 0]
        kf, kl = int(real[0]), int(real[-1])
        wins = list(range((kf // WIN) * WIN, kl + 1, WIN))
        for wi, w in enumerate(wins):
            ind = (cid_pm[c][:, None] ==
                   (w + np.arange(WIN))[None, :]).astype(ml_dtypes.float8_e4m3)
            if wi == 0:
                oh[c] = ind
                ref = (c, 0, c)
            else:
                oh2_list.append(ind)
                ref = (c, 1, len(oh2_list) - 1)
            touch.setdefault(w, []).append(ref)
    # every 32-window of both halves must be touched so psum gets start=True
    # coverage before the raw-sum copy reads the full bank
    for w in range(0, CLUSTER, WIN):
        if w not in touch:
            oh2_list.append(np.zeros((128, WIN), ml_dtypes.float8_e4m3))
            touch[w] = [(0, 1, len(oh2_list) - 1)]
    emit = [[] for _ in range(KCHUNKS)]
    for w, refs in touch.items():
        h, base = w // 128, w % 128
        for k, (c, src, idx) in enumerate(refs):
            emit[c].append((src, idx, h, base,
                            k == 0, k == len(refs) - 1))
    emit = tuple(tuple(sorted(e)) for e in emit)
    oh = np.ascontiguousarray(oh.transpose(1, 0, 2))       # [128, c, 32]
    ncross = max(1, len(oh2_list))
    oh2 = np.zeros((128, ncross, WIN), ml_dtypes.float8_e4m3)
    for i, ind in enumerate(oh2_list):
        oh2[:, i] = ind

    # batch-cluster onehots for the pos/neg center gather matmuls
    nbpc = BATCH // NCORES
    cpos = cluster_ids[pos]
    cneg = cluster_ids[neg]
    prow = np.arange(128)
    counts = np.bincount(cluster_ids, minlength=CLUSTER).astype(np.float64)

    shared = {
        "t2i": t2i,
        "oh": oh,
        "oh2": oh2,
        "ut_bf": user_table.astype(ml_dtypes.bfloat16),
        "it_bf": item_table.astype(ml_dtypes.bfloat16),
    }
    in_maps = []
    num_rel = np.zeros((upad,), np.float64)
    for ci in range(NCORES):
        rows = uu[ci * upc: (ci + 1) * upc]
        gathered = train_label[rows]          # [upc, 20001] f32
        num_rel[ci * upc: (ci + 1) * upc] = gathered.sum(axis=1)
        lt = np.zeros((KPAD, upc), ml_dtypes.float8_e4m3)
        lt[: NUM_ITEMS + 1] = gathered.T[perm].astype(ml_dtypes.float8_e4m3)
        lt[q] = 0                             # uni excludes item 20000
        ltpm = np.ascontiguousarray(
            lt.reshape(KCHUNKS, 128, upc).transpose(1, 0, 2)
        )
        bs = slice(ci * nbpc, (ci + 1) * nbpc)
        ohb = np.zeros((128, 4, nbpc), ml_dtypes.float8_e4m3)
        ohb[:, 0] = (cpos[bs][None, :] == prow[:, None])
        ohb[:, 1] = (cpos[bs][None, :] == (prow + 128)[:, None])
        ohb[:, 2] = (cneg[bs][None, :] == prow[:, None])
        ohb[:, 3] = (cneg[bs][None, :] == (prow + 128)[:, None])
        m = dict(shared)
        m["lt"] = ltpm
        m["ohb"] = ohb
        m["uidx"] = _wrap_idx(user[bs])
        m["pnidx"] = np.concatenate(
            [_wrap_idx(pos[bs]), _wrap_idx(neg[bs])], axis=1
        )
        in_maps.append(m)

    meta = {"upc": upc, "nbpc": nbpc, "nu": nu, "inverse": inverse,
            "emit": emit, "ncross": ncross, "num_rel": num_rel,
            "counts": counts, "cpos": cpos, "cneg": cneg}
    return in_maps, meta


def _unshuffle_pm(arr):
    """[128, nblk, 64] partition-major -> [nblk*128, 64] row-major f32."""
    arr = np.asarray(arr, dtype=np.float32)
    return np.ascontiguousarray(arr.transpose(1, 0, 2)).reshape(-1, arr.shape[2])


def assemble(results, meta):
    inverse = meta["inverse"]
    upc = meta["upc"]
    nbpc = meta["nbpc"]
    uni_raw = np.concatenate(
        [_unshuffle_pm(r["uni_part"])[:upc] for r in results], axis=0
    )
    uni_unique = uni_raw / meta["num_rel"][: len(uni_raw), None]
    uni = uni_unique[inverse].astype(np.float32)
    ue = np.concatenate([_unshuffle_pm(r["ue_out"]) for r in results], axis=0)
    pne = [
        _unshuffle_pm(r["pne_out"]).reshape(2, nbpc, DIM) for r in results
    ]
    pe = np.concatenate([x[0] for x in pne], axis=0)
    ne = np.concatenate([x[1] for x in pne], axis=0)
    pc_raw = np.concatenate(
        [_unshuffle_pm(r["pct_out"].reshape(128, -1, DIM)) for r in results],
        axis=0,
    )
    nc_raw = np.concatenate(
        [_unshuffle_pm(r["nct_out"].reshape(128, -1, DIM)) for r in results],
        axis=0,
    )
    counts = meta["counts"]
    pc = (pc_raw / np.maximum(counts[meta["cpos"]], 1.0)[:, None]).astype(np.float32)
    ncen = (nc_raw / np.maximum(counts[meta["cneg"]], 1.0)[:, None]).astype(np.float32)
    return ue, pe, ne, pc, ncen, uni


_CACHE = {}


def build_from_meta(meta):
    return build_bass(meta["upc"], meta["nbpc"], meta["emit"], meta["ncross"])


def _run(in_maps, meta, trace=False):
    from concourse.bass_utils import run_bass_kernel_spmd

    key = (meta["upc"], meta["nbpc"], meta["emit"], meta["ncross"])
    if key not in _CACHE:
        _CACHE[key] = build_from_meta(meta)
    nc = _CACHE[key]
    res = run_bass_kernel_spmd(
        nc, in_maps, core_ids=list(range(NCORES)), trace=trace
    )
    return res


def kernel(user, pos, neg, cluster_ids, user_table, item_table, train_label):
    """Full (unsharded) inputs -> full outputs, computed on 8 NeuronCores."""
    in_maps, meta = host_prep(
        user, pos, neg, cluster_ids, user_table, item_table, train_label
    )
    res = _run(in_maps, meta)
    return assemble(res.results, meta)
